# revision 1
# baseline (speedup 1.0000x reference)
"""AutoCorrelation Trainium2 kernel (Bass/Tile, 8 NeuronCores).

Math (per row r of [B*L, 512] with D=512):
  corr_r = irfft(rfft(q_r) * conj(rfft(k_r)))            (circular cross-correlation)
  mean_r = mean(top7(corr_r))
  w0 = sigmoid(corr - mean); out = v*w0 + roll(v,-1,L)*(1-w0)
     = v + sigmoid(mean - corr) * (roll(v) - v)

Implementation:
  - DFT/iDFT as fp16 matmuls on the PE with a packed-real 512-point basis:
    packed[f] layout: A-block f=0..255 = Re[f] (A[0]=Re0), B-block = Im[f]
    (B[0]=Re256).  Forward: QF^T[fpacked, row] = W^T q^T via
    lhsT=W-block, rhs=qT (DMA-xbar-transposed q16).  Product spectrum
    P = QF o conj(KF) elementwise on DVE (block formulas + 2 f=0 fixups).
    Inverse: corr[row, t] via lhsT=P-chunk, rhs=C-block -> PSUM fp32,
    already in row-major layout.
  - top-7 mean via the DVE max8 instruction reading corr in PSUM.
  - sigmoid on ACT directly off PSUM with per-partition bias = +mean/scale=-1.
  - Row interleave: partition p = row//64, subblock s = row%64 makes
    roll(v,-1) = "read subblock s+1" (same partitions); batch wraps and the
    s=63 edge are handled by one small shifted DMA load (vsh).
  - Sharding: batch-parallel, 4 batches per core, no communication.
"""
import numpy as np

B, L, D = 32, 2048, 512
N_CORES = 8
BPC = B // N_CORES            # batches per core
ROWS = BPC * L                # 8192 rows per core
NSUB = 64                     # subblocks (s = row % 64)
P = 128                       # partitions (p = row // 64)
SB_GROUP = 8                  # subblocks per DMA superblock
NSUPER = NSUB // SB_GROUP     # 8 superblocks
TOPK = 7

_CACHE = {}


def _dft_consts():
    """Packed-real DFT matrices W [512 feat, 512 packed] and C [512 packed, 512 t]."""
    j = np.arange(D)[:, None].astype(np.float64)
    f = np.arange(256)[None, :].astype(np.float64)
    Wc = np.cos(-2 * np.pi * j * f / D)
    Ws = np.sin(-2 * np.pi * j * f / D)
    WB = Ws.copy()
    WB[:, 0] = np.cos(np.pi * j[:, 0])          # B0 row: Re256
    W = np.concatenate([Wc, WB], axis=1)        # [512, 512]
    t = np.arange(D)[None, :].astype(np.float64)
    fc = np.arange(256)[:, None].astype(np.float64)
    Ca = np.cos(2 * np.pi * fc * t / D) * 2 / D
    Ca[0] = 1.0 / D
    Cb = -np.sin(2 * np.pi * fc * t / D) * 2 / D
    Cb[0] = np.cos(np.pi * t[0]) / D
    C = np.concatenate([Ca, Cb], axis=0)        # [512, 512]
    return W.astype(np.float32), C.astype(np.float32)


def _build_nc(n_iter=1):
    import os
    import concourse.bacc as bacc
    import concourse.mybir as mybir
    from concourse.tile import TileContext

    ABL = set(os.environ.get("AUTOCORR_ABL", "").split(","))

    f16 = mybir.dt.float16
    f32 = mybir.dt.float32

    W, C = _dft_consts()
    # W16[p, jj, fp]  = W[jj*128+p, fp]   (lhsT blocks for GEMM-1)
    W16 = W.reshape(4, P, D).transpose(1, 0, 2).astype(np.float16).copy()
    # C16[p, ff, t]   = C[ff*128+p, t]    (rhs blocks for GEMM-2)
    C16 = C.reshape(4, P, D).transpose(1, 0, 2).astype(np.float16).copy()

    nc = bacc.Bacc()
    q_d = nc.dram_tensor("query", [ROWS, D], f32, kind="ExternalInput")
    k_d = nc.dram_tensor("key", [ROWS, D], f32, kind="ExternalInput")
    v_d = nc.dram_tensor("value", [ROWS, D], f32, kind="ExternalInput")
    o_d = nc.dram_tensor("out", [ROWS, D], f32, kind="ExternalOutput")
    w_t = nc.inline_tensor(W16, name="Wdft")
    c_t = nc.inline_tensor(C16, name="Cdft")

    # interleaved views: [p, s, c] with row = 64*p + s
    qv = q_d.rearrange("(p s) c -> p s c", s=NSUB)
    kv = k_d.rearrange("(p s) c -> p s c", s=NSUB)
    vv = v_d.rearrange("(p s) c -> p s c", s=NSUB)
    ov = o_d.rearrange("(p s) c -> p s c", s=NSUB)

    with TileContext(nc) as tc:
        with (
            tc.tile_pool(name="consts", bufs=1) as consts,
            tc.tile_pool(name="io", bufs=2) as io,
            tc.tile_pool(name="work", bufs=3) as work,
            tc.tile_pool(name="small", bufs=8) as small,
            tc.tile_pool(name="ps", bufs=3, space="PSUM") as psp,
            tc.tile_pool(name="pscb", bufs=2, space="PSUM") as pscp,
        ):
            wt = consts.tile([P, 4, D], f16)      # W16
            ct = consts.tile([P, 4, D], f16)      # C16
            nc.sync.dma_start(out=wt, in_=w_t[:, :, :])
            nc.sync.dma_start(out=ct, in_=c_t[:, :, :])

            # vsh[p] = v[row 64p+64] ; fix wraps at p in {31,63,95,127} <- batch starts
            vsh = consts.tile([P, D], f16)
            vflat = v_d  # [ROWS, D]
            nc.gpsimd.dma_start(
                out=vsh[0:127], in_=vflat.rearrange("(a b) c -> a b c", b=NSUB)[1:128, 0]
            )  # rows 64,128,...,8128
            nc.gpsimd.dma_start(
                out=vsh.rearrange("(w u) c -> w u c", u=32)[:, 31:32, :].rearrange("w u c -> (w u) c"),
                in_=vflat.rearrange("(b t) c -> b t c", t=L)[:, 0:1, :].rearrange("b t c -> (b t) c"),
            )  # vsh[31,63,95,127] <- v rows {0, 2048, 4096, 6144}

            def load_super(sbi):
                sl = slice(sbi * SB_GROUP, (sbi + 1) * SB_GROUP)
                q16 = io.tile([P, SB_GROUP, D], f16, tag="q16")
                k16 = io.tile([P, SB_GROUP, D], f16, tag="k16")
                v16 = io.tile([P, SB_GROUP, D], f16, tag="v16")
                if "loadhalf" in ABL:
                    nc.gpsimd.dma_start(out=q16, in_=qv[:, sl, :])
                    return q16, q16, q16
                nc.gpsimd.dma_start(out=q16, in_=qv[:, sl, :])
                nc.gpsimd.dma_start(out=k16, in_=kv[:, sl, :])
                nc.gpsimd.dma_start(out=v16, in_=vv[:, sl, :])
                return q16, k16, v16

            def compute_group(qT8, kT8, gl, w1sb):
                """gl: local group index (0..3) inside superblock; reads subblocks
                2gl, 2gl+1 from the whole-superblock transpose tiles qT8/kT8
                (mid index u*4+jj, u = local subblock); writes w1 into w1sb."""
                psq = psp.tile([P, 4, 256], f32, tag="ps2bank")
                psk = psp.tile([P, 4, 256], f32, tag="ps2bank")
                x0 = 8 * gl
                for mm in range(4):
                    for jj in range(4):
                        nc.tensor.matmul(psq[:, mm, :], wt[:, jj, mm * P:(mm + 1) * P],
                                         qT8[:, x0 + jj:x0 + jj + 5:4, :],
                                         start=(jj == 0), stop=(jj == 3))
                for mm in range(4):
                    for jj in range(4):
                        nc.tensor.matmul(psk[:, mm, :], wt[:, jj, mm * P:(mm + 1) * P],
                                         kT8[:, x0 + jj:x0 + jj + 5:4, :],
                                         start=(jj == 0), stop=(jj == 3))

                qf = work.tile([P, 4, 256], f16, tag="qf")
                kf = work.tile([P, 4, 256], f16, tag="kf")
                nc.scalar.copy(qf, psq)
                nc.scalar.copy(kf, psk)

                # products: Pa = QA.KA + QB.KB ; Pb = QB.KA - QA.KB
                pt = work.tile([P, 4, 256], f16, tag="pt")
                t1 = work.tile([P, 2, 256], f16, tag="t1")
                t2 = work.tile([P, 2, 256], f16, tag="t2")
                QA, QB = qf[:, 0:2, :], qf[:, 2:4, :]
                KA, KB = kf[:, 0:2, :], kf[:, 2:4, :]
                nc.vector.tensor_mul(t1, QA, KA)
                nc.vector.tensor_mul(t2, QB, KB)
                nc.vector.tensor_add(pt[:, 0:2, :], t1, t2)
                nc.vector.tensor_mul(t1, QB, KA)
                nc.vector.tensor_mul(t2, QA, KB)
                nc.vector.tensor_sub(pt[:, 2:4, :], t1, t2)
                # f=0 fixups (partition 0 of slices 0 and 2), one strided op
                nc.vector.tensor_mul(
                    pt[0:1, 0:4:2, :], qf[0:1, 0:4:2, :], kf[0:1, 0:4:2, :])

                for sp in range(2):
                    cps = pscp.tile([P, D], f32, tag="psc1bank")
                    for ff in range(4):
                        nc.tensor.matmul(cps, pt[:, ff, sp * P:(sp + 1) * P],
                                         ct[:, ff, :], start=(ff == 0), stop=(ff == 3))
                    mx = small.tile([P, 8], f32, tag="mx")
                    nc.vector.max(out=mx, in_=cps)
                    sm = small.tile([P, 1], f32, tag="sm")
                    nc.vector.reduce_sum(sm, mx[:, 0:TOPK], axis=mybir.AxisListType.X)
                    pm = small.tile([P, 1], f32, tag="pm")
                    nc.vector.tensor_scalar_mul(pm, sm, 1.0 / TOPK)
                    nc.scalar.activation(w1sb[:, 2 * gl + sp, :], cps,
                                         mybir.ActivationFunctionType.Sigmoid,
                                         bias=pm, scale=-1.0)

            def combine_super(v16, w1sb, vnext0, o16):
                """o16[:, s] = v16[:, s] + w1sb[:, s]*(v16[:, s+1] - v16[:, s]);
                s=7 uses vnext0."""
                for sl in range(SB_GROUP):
                    vnext = v16[:, sl + 1, :] if sl < SB_GROUP - 1 else vnext0
                    dt_ = work.tile([P, D], f16, tag="dt")
                    zt = work.tile([P, D], f16, tag="zt")
                    nc.vector.tensor_sub(dt_, vnext, v16[:, sl, :])
                    nc.vector.tensor_mul(zt, w1sb[:, sl, :], dt_)
                    nc.gpsimd.tensor_add(o16[:, sl, :], v16[:, sl, :], zt)

            def pipeline():
                prev = None  # (v16, o16, w1sb, sbi)
                for sbi in range(NSUPER):
                    q16, k16, v16 = load_super(sbi)
                    o16 = io.tile([P, SB_GROUP, D], f16, tag="o16")
                    w1sb = work.tile([P, SB_GROUP, D], f16, tag="w1sb", bufs=2)
                    qT8 = work.tile([P, 32, P], f16, tag="qT8", bufs=2)
                    kT8 = work.tile([P, 32, P], f16, tag="kT8", bufs=2)
                    nc.sync.dma_start_transpose(
                        qT8, q16.rearrange("p s c -> p (s c)"))
                    nc.sync.dma_start_transpose(
                        kT8, k16.rearrange("p s c -> p (s c)"))
                    for gl in range(4):
                        compute_group(qT8, kT8, gl, w1sb)
                    if prev is not None:
                        pv, po, pw, psbi = prev
                        combine_super(pv, pw, v16[:, 0, :], po)
                        nc.gpsimd.dma_start(
                            out=ov[:, psbi * SB_GROUP:(psbi + 1) * SB_GROUP, :], in_=po)
                    prev = (v16, o16, w1sb, sbi)

                pv, po, pw, psbi = prev
                combine_super(pv, pw, vsh, po)
                nc.gpsimd.dma_start(
                    out=ov[:, psbi * SB_GROUP:(psbi + 1) * SB_GROUP, :], in_=po)

            if n_iter == 1:
                pipeline()
            else:
                with tc.For_i(0, n_iter, 1):
                    pipeline()

    nc.finalize()
    return nc


def kernel(query, key, value):
    import sys
    if "/opt/trn_rl_repo" not in sys.path:
        sys.path.insert(0, "/opt/trn_rl_repo")
    from concourse.bass_utils import run_bass_kernel_spmd

    if "nc" not in _CACHE:
        _CACHE["nc"] = _build_nc()
    nc = _CACHE["nc"]

    q = np.ascontiguousarray(np.asarray(query, dtype=np.float32).reshape(B, L, D))
    k = np.ascontiguousarray(np.asarray(key, dtype=np.float32).reshape(B, L, D))
    v = np.ascontiguousarray(np.asarray(value, dtype=np.float32).reshape(B, L, D))

    in_maps = []
    for c in range(N_CORES):
        sl = slice(c * BPC, (c + 1) * BPC)
        in_maps.append({
            "query": q[sl].reshape(ROWS, D),
            "key": k[sl].reshape(ROWS, D),
            "value": v[sl].reshape(ROWS, D),
        })
    res = run_bass_kernel_spmd(nc, in_maps, core_ids=list(range(N_CORES)),
                               trace=bool(_CACHE.get("trace")))
    _CACHE["last_result"] = res
    out = np.empty((B, L, D), dtype=np.float32)
    for c in range(N_CORES):
        out[c * BPC:(c + 1) * BPC] = res.results[c]["out"].reshape(BPC, L, D)
    return out



# revision 4
# speedup vs baseline: 10.6759x; 10.6759x over previous
"""AutoCorrelation Trainium2 kernel (Bass/Tile, 8 NeuronCores) — v2.

Math (per row r of [B*L, 512] with D=512):
  corr_r = irfft(rfft(q_r) * conj(rfft(k_r)))            (circular cross-correlation)
  mean_r = mean(top7(corr_r))
  out = v + sigmoid(mean - corr) * (roll(v,-1,L) - v)

Implementation notes:
  - Host casts q/k/v to fp16 before upload and the kernel emits an fp16
    output (cast back to fp32 on host): HBM traffic 33 MiB/core instead of
    64, and every DMA is HWDGE (no SWDGE casts) so GpSimd is free for
    elementwise work.
  - DFT/iDFT as fp16 matmuls with a packed-real 512-point basis
    (A-block f=0..255 = Re[f] with A[0]=Re0, B-block = Im[f] with
    B[0]=Re256).  Forward rhs comes from an xbar DMA-transpose pulled
    STRAIGHT from DRAM.  q/k share each W-block LDWEIGHTS (interleaved).
  - Product spectrum on DVE fp16 (2x_1P mode, no Pool-port contention),
    superblock-wide FD=2048 ops + one f=0 fixup op.
  - Inverse GEMM accumulates into PSUM with C pre-scaled by 1/7, so
    reduce_sum(top7) IS the top-k mean; ACT sigmoid(bias=mean, scale=-7)
    reads PSUM directly.
  - Row interleave: partition p = row//64, subblock s = row%64 so
    roll(v,-1) = "read subblock s+1"; v is loaded 9 subblocks per
    8-subblock superblock (vnext = v9[:,1:9]); the last superblock's 9th
    slot is filled by a strided row-64k load + 4 batch-wrap rows.
  - Sharding: batch-parallel, 4 batches per core, no communication.
"""
import numpy as np

B, L, D = 32, 2048, 512
N_CORES = 8
BPC = B // N_CORES            # batches per core
ROWS = BPC * L                # 8192 rows per core
NSUB = 64                     # subblocks (s = row % 64)
P = 128                       # partitions (p = row // 64)
SB_GROUP = 8                  # subblocks per superblock
NSUPER = NSUB // SB_GROUP     # 8 superblocks
TOPK = 7

_CACHE = {}


def _dft_consts():
    """Packed-real DFT matrices W [512 feat, 512 packed] and C [512 packed, 512 t].
    C is pre-scaled by 1/TOPK so sum(top7(corr')) == mean(top7(corr))."""
    j = np.arange(D)[:, None].astype(np.float64)
    f = np.arange(256)[None, :].astype(np.float64)
    Wc = np.cos(-2 * np.pi * j * f / D)
    Ws = np.sin(-2 * np.pi * j * f / D)
    WB = Ws.copy()
    WB[:, 0] = np.cos(np.pi * j[:, 0])          # B0 row: Re256
    W = np.concatenate([Wc, WB], axis=1)        # [512, 512]
    t = np.arange(D)[None, :].astype(np.float64)
    fc = np.arange(256)[:, None].astype(np.float64)
    Ca = np.cos(2 * np.pi * fc * t / D) * 2 / D
    Ca[0] = 1.0 / D
    Cb = -np.sin(2 * np.pi * fc * t / D) * 2 / D
    Cb[0] = np.cos(np.pi * t[0]) / D
    C = np.concatenate([Ca, Cb], axis=0) / TOPK  # [512, 512]
    return W.astype(np.float32), C.astype(np.float32)


def _build_nc(n_iter=1):
    import concourse.bacc as bacc
    import concourse.mybir as mybir
    from concourse.tile import TileContext

    f16 = mybir.dt.float16
    f32 = mybir.dt.float32

    W, C = _dft_consts()
    # W16[p, jj, fp]  = W[jj*128+p, fp]   (lhsT blocks for GEMM-1)
    W16 = W.reshape(4, P, D).transpose(1, 0, 2).astype(np.float16).copy()
    # C16[p, ff, t]   = C[ff*128+p, t]    (rhs blocks for GEMM-2)
    C16 = C.reshape(4, P, D).transpose(1, 0, 2).astype(np.float16).copy()

    nc = bacc.Bacc()
    q_d = nc.dram_tensor("query", [ROWS, D], f16, kind="ExternalInput")
    k_d = nc.dram_tensor("key", [ROWS, D], f16, kind="ExternalInput")
    v_d = nc.dram_tensor("value", [ROWS, D], f16, kind="ExternalInput")
    o_d = nc.dram_tensor("out", [ROWS, D], f16, kind="ExternalOutput")
    w_t = nc.inline_tensor(W16, name="Wdft")
    c_t = nc.inline_tensor(C16, name="Cdft")

    # interleaved views: [p, s, c] with row = 64*p + s
    qv = q_d.rearrange("(p s) c -> p s c", s=NSUB)
    kv = k_d.rearrange("(p s) c -> p s c", s=NSUB)
    vv = v_d.rearrange("(p s) c -> p s c", s=NSUB)
    ov = o_d.rearrange("(p s) c -> p s c", s=NSUB)

    with TileContext(nc) as tc:
        with (
            tc.tile_pool(name="consts", bufs=1) as consts,
            tc.tile_pool(name="io", bufs=2) as io,
            tc.tile_pool(name="work", bufs=2) as work,
            tc.tile_pool(name="small", bufs=8) as small,
            tc.tile_pool(name="ps", bufs=3, space="PSUM") as psp,
            tc.tile_pool(name="pscb", bufs=2, space="PSUM") as pscp,
        ):
            wt = consts.tile([P, 4, D], f16)      # W16
            ct = consts.tile([P, 4, D], f16)      # C16
            nc.sync.dma_start(out=wt, in_=w_t[:, :, :])
            nc.sync.dma_start(out=ct, in_=c_t[:, :, :])

            def superblock(sbi):
                sl = slice(sbi * SB_GROUP, (sbi + 1) * SB_GROUP)
                qT8 = work.tile([P, 32, P], f16, tag="qT8")
                kT8 = work.tile([P, 32, P], f16, tag="kT8")
                nc.sync.dma_start_transpose(
                    qT8, qv[:, sl, :].rearrange("p s c -> p (s c)"))
                nc.sync.dma_start_transpose(
                    kT8, kv[:, sl, :].rearrange("p s c -> p (s c)"))

                v9 = io.tile([P, SB_GROUP + 1, D], f16, tag="v9")
                if sbi < NSUPER - 1:
                    nc.sync.dma_start(
                        out=v9, in_=vv[:, sbi * SB_GROUP:(sbi + 1) * SB_GROUP + 1, :])
                else:
                    nc.sync.dma_start(out=v9[:, 0:SB_GROUP, :], in_=vv[:, sl, :])
                    # v9[p, 8] = v[row 64p+64]; wraps at p in {31,63,95,127}
                    nc.sync.dma_start(
                        out=v9[0:127, SB_GROUP, :],
                        in_=v_d.rearrange("(a b) c -> a b c", b=NSUB)[1:128, 0])
                    nc.sync.dma_start(
                        out=v9.rearrange("(w u) s c -> w u s c", u=32)[:, 31, SB_GROUP, :],
                        in_=v_d.rearrange("(b t) c -> b t c", t=L)[:, 0, :])

                # forward DFT: psq/psk [freq-chunk mm, rows], q/k share LDWEIGHTS
                qf = work.tile([P, 4, 4 * 256], f16, tag="qf")
                kf = work.tile([P, 4, 4 * 256], f16, tag="kf")
                for gl in range(4):
                    psq = psp.tile([P, 4, 256], f32, tag="ps2bank")
                    psk = psp.tile([P, 4, 256], f32, tag="ps2bank")
                    x0 = 8 * gl
                    for mm in range(4):
                        for jj in range(4):
                            lw = wt[:, jj, mm * P:(mm + 1) * P]
                            rq = qT8[:, x0 + jj:x0 + jj + 5:4, :]
                            rk = kT8[:, x0 + jj:x0 + jj + 5:4, :]
                            nc.tensor.matmul(psq[:, mm, :], lw, rq,
                                             start=(jj == 0), stop=(jj == 3))
                            nc.tensor.matmul(psk[:, mm, :], lw, rk,
                                             start=(jj == 0), stop=(jj == 3))
                    nc.scalar.copy(qf[:, :, gl * 256:(gl + 1) * 256], psq)
                    nc.scalar.copy(kf[:, :, gl * 256:(gl + 1) * 256], psk)

                # product spectrum P = QF o conj(KF), superblock-wide on DVE
                pt = work.tile([P, 4, 1024], f16, tag="pt")
                t1 = work.tile([P, 2, 1024], f16, tag="t1")
                t2 = work.tile([P, 2, 1024], f16, tag="t2")
                QA, QB = qf[:, 0:2, :], qf[:, 2:4, :]
                KA, KB = kf[:, 0:2, :], kf[:, 2:4, :]
                nc.vector.tensor_mul(t1, QA, KA)
                nc.vector.tensor_mul(t2, QB, KB)
                nc.vector.tensor_add(pt[:, 0:2, :], t1, t2)
                nc.vector.tensor_mul(t1, QB, KA)
                nc.vector.tensor_mul(t2, QA, KB)
                nc.vector.tensor_sub(pt[:, 2:4, :], t1, t2)
                # f=0 fixup (partition 0 of slices 0 and 2), one strided op
                nc.vector.tensor_mul(
                    pt[0:1, 0:4:2, :], qf[0:1, 0:4:2, :], kf[0:1, 0:4:2, :])

                # inverse DFT per 128-row chunk (= subblock), then w1 weights
                w1sb = work.tile([P, SB_GROUP, D], f16, tag="w1sb")
                for ch in range(SB_GROUP):
                    cps = pscp.tile([P, D], f32, tag="psc1bank")
                    for ff in range(4):
                        nc.tensor.matmul(cps, pt[:, ff, ch * P:(ch + 1) * P],
                                         ct[:, ff, :], start=(ff == 0), stop=(ff == 3))
                    mx = small.tile([P, 8], f32, tag="mx")
                    nc.vector.max(out=mx, in_=cps)
                    pm = small.tile([P, 1], f32, tag="pm")
                    nc.vector.reduce_sum(pm, mx[:, 0:TOPK],
                                         axis=mybir.AxisListType.X)
                    nc.scalar.activation(w1sb[:, ch, :], cps,
                                         mybir.ActivationFunctionType.Sigmoid,
                                         bias=pm, scale=-float(TOPK))

                # out = v + w1*(vnext - v): sub on DVE, mul+add on GpSimd
                dt_ = work.tile([P, SB_GROUP, D], f16, tag="dt")
                zt = work.tile([P, SB_GROUP, D], f16, tag="zt")
                o16 = io.tile([P, SB_GROUP, D], f16, tag="o16")
                nc.vector.tensor_sub(dt_, v9[:, 1:SB_GROUP + 1, :],
                                     v9[:, 0:SB_GROUP, :])
                nc.gpsimd.tensor_mul(zt, w1sb, dt_)
                nc.gpsimd.tensor_add(o16, v9[:, 0:SB_GROUP, :], zt)
                # ACT ring: keeps the store off the SP ring so next-superblock
                # loads never queue behind it (HWDGE rings are FIFO)
                nc.scalar.dma_start(out=ov[:, sl, :], in_=o16)

            def pipeline():
                for sbi in range(NSUPER):
                    superblock(sbi)

            if n_iter == 1:
                pipeline()
            else:
                with tc.For_i(0, n_iter, 1):
                    pipeline()

    nc.finalize()
    return nc


def kernel(query, key, value):
    import sys
    if "/opt/trn_rl_repo" not in sys.path:
        sys.path.insert(0, "/opt/trn_rl_repo")
    from concourse.bass_utils import run_bass_kernel_spmd

    if "nc" not in _CACHE:
        _CACHE["nc"] = _build_nc()
    nc = _CACHE["nc"]

    q = np.asarray(query, dtype=np.float32).reshape(B, L, D).astype(np.float16)
    k = np.asarray(key, dtype=np.float32).reshape(B, L, D).astype(np.float16)
    v = np.asarray(value, dtype=np.float32).reshape(B, L, D).astype(np.float16)

    in_maps = []
    for c in range(N_CORES):
        sl = slice(c * BPC, (c + 1) * BPC)
        in_maps.append({
            "query": np.ascontiguousarray(q[sl].reshape(ROWS, D)),
            "key": np.ascontiguousarray(k[sl].reshape(ROWS, D)),
            "value": np.ascontiguousarray(v[sl].reshape(ROWS, D)),
        })
    res = run_bass_kernel_spmd(nc, in_maps, core_ids=list(range(N_CORES)),
                               trace=bool(_CACHE.get("trace")))
    _CACHE["last_result"] = res
    out = np.empty((B, L, D), dtype=np.float32)
    for c in range(N_CORES):
        out[c * BPC:(c + 1) * BPC] = res.results[c]["out"].astype(
            np.float32).reshape(BPC, L, D)
    return out
